# revision 33
# baseline (speedup 1.0000x reference)
"""Non-local block (no softmax) on 8 Trainium2 cores, data-parallel over batch.

Math: per sample X [N=4096, C=256] (N = 64*64 spatial, C channels):
    theta = X Wt, phi = X Wp, g = X Wg          (biases are zero)
    y = (theta phi^T / N) g  ->  associativity (no softmax):
      y = X L G R,   L = Wt Wp^T,  R = Wg (Ww*s) / N,  G = X^T X
    z = y + t2 + X,  s = gamma*rsqrt(var+eps),  t2 = (b_W - mean)*s + beta
Device computes delta^T = (L G R)^T X^T + t2 in bf16; host adds X (f32).

With all 8 cores running, the kernel is HBM-wire-bound: ~5.3MB/core of
traffic paces everything.  v5 therefore packs EVERY transfer as a fully
contiguous DRAM block (one tensor per transfer, host pre/post-packs) so
HBM sees sequential streams, keeps the transfer count minimal, spreads
the load over 3 queues (sync/scalar HWDGE + gpsimd SWDGE), uses the
symmetry of G (384 instead of 512 PSUM columns/chunk + one PE
transpose), and bridges the HAM cold window with dummy matmuls.
"""

import numpy as np
import ml_dtypes

B, H, W, C = 8, 64, 64, 256
IC = C // 2
N = H * W
NCHUNK = N // 128  # 32
BN_EPS = 1e-3

_CACHE = {}
DEFAULT_MODE = "v9"
N_WARMUP = 4
# x8 pieces (chunk ranges) in consumption order, all on the sync queue:
# concurrent DMA queues interleave packets and wreck HBM streaming (2MB
# on 1 queue costs +2.4us vs +5.1us on 2 queues, measured), so a single
# queue carries everything in consumption order.
XSPLIT = [(0, 2, 0), (2, 4, 0), (6, 8, 0), (14, 9, 0), (23, 9, 0)]


def _build_nc(mode: str):
    import concourse.bacc as bacc
    import concourse.mybir as mybir
    import concourse.tile as tile

    F32 = mybir.dt.float32
    BF16 = mybir.dt.bfloat16
    FP8 = mybir.dt.float8e4

    nc = bacc.Bacc("TRN2", target_bir_lowering=False, debug=False)

    # one contiguous DRAM tensor per transfer
    x8_d = [nc.dram_tensor(f"x8_{u}", [128, 256 * n], FP8,
                           kind="ExternalInput")
            for u, (c0, n, r) in enumerate(XSPLIT)]
    # xq_d[q][p, k*1024+j] = X^T[k*128+p, q*1024+j]
    xq_d = [nc.dram_tensor(f"xq_{q}", [128, 2048], BF16,
                           kind="ExternalInput") for q in range(4)]
    # wl: [0:512] R-pack, [512:1024] L^T-pack, [1024:1152] identity
    wl_d = nc.dram_tensor("wl", [128, 1152], BF16, kind="ExternalInput")
    t2c_d = nc.dram_tensor("t2c", [128, 2], F32, kind="ExternalInput")
    # output: one contiguous chunk per (q, m): delta^T[m-rows, q-cols]
    dt_d = [nc.dram_tensor(f"dt_{q}{m}", [128, 1024], BF16,
                           kind="ExternalOutput")
            for q in range(4) for m in range(2)]

    with tile.TileContext(nc) as tc:
        with (
            tc.tile_pool(name="const", bufs=1) as cpool,
            tc.tile_pool(name="big", bufs=1) as bpool,
            tc.tile_pool(name="psg", bufs=1, space="PSUM") as psg,
            tc.tile_pool(name="psz", bufs=3, space="PSUM") as psz,
        ):
            wl = cpool.tile([128, 1152], BF16, tag="wl")
            t2c = cpool.tile([128, 2], F32, tag="t2c")
            wz = cpool.tile([128, 512], BF16, tag="wz")
            # gpsimd is otherwise idle and starts earliest
            nc.gpsimd.memset(wz[:], 0.0)

            x8_t = [bpool.tile([128, 256 * n], FP8, tag=f"x8_{u}",
                               name=f"x8_{u}")
                    for u, (c0, n, r) in enumerate(XSPLIT)]
            xq = [bpool.tile([128, 2048], BF16, tag=f"xq{q}",
                             name=f"xq{q}") for q in range(4)]

            # ---- input DMAs: ALL on the sync queue, consumption order
            for u, (c0, n, r) in enumerate(XSPLIT):
                nc.sync.dma_start(x8_t[u][:], x8_d[u][:])
            nc.sync.dma_start(xq[0][:], xq_d[0][:])
            nc.sync.dma_start(wl[:], wl_d[:])
            for q in range(1, 4):
                nc.sync.dma_start(xq[q][:], xq_d[q][:])
            nc.sync.dma_start(t2c[:], t2c_d[:])

            # dummies bridge PE from preamble end to the first x8 piece
            # landing so HAM warmth builds with no idle gap
            wu = psz.tile([128, 1024], F32, tag="z")
            for _ in range(N_WARMUP):
                nc.tensor.matmul(wu[:, 0:512], wz[:, 0:128], wz[:],
                                 start=True, stop=True, skip_group_check=True)

            # ---- phase 1: G = X^T X (fp8, streams behind the x8 DMAs).
            # By symmetry only G[0:128, 0:256] (-> gps[:,0:256]) and
            # G[128:256, 128:256] (-> gps[:,256:384]) are computed; G21
            # comes from a PE transpose of G12.  The row blocks share
            # one PSUM bank: the first matmul's start=True clears
            # has_written for the whole bank, the second group opens
            # with start=False (overwrite where has_written==0).
            gps = psg.tile([128, 512], F32, tag="gp")
            gt_ps = psg.tile([128, 128], BF16, tag="gt")
            g_s = [bpool.tile([128, 256], BF16, tag=f"g_s{j}", name=f"g_s{j}")
                   for j in range(2)]
            for t in range(NCHUNK):
                u = next(i for i, (c0, n, r) in enumerate(XSPLIT)
                         if c0 <= t < c0 + n)
                xn = x8_t[u]
                o = (t - XSPLIT[u][0]) * 256
                nc.tensor.matmul(gps[:, 0:256], xn[:, o:o + 128],
                                 xn[:, o:o + 256],
                                 start=(t == 0), stop=(t == NCHUNK - 1),
                                 skip_group_check=True)
                nc.tensor.matmul(gps[:, 256:384], xn[:, o + 128:o + 256],
                                 xn[:, o + 128:o + 256],
                                 start=False, stop=(t == NCHUNK - 1),
                                 skip_group_check=True)

            # ---- phase 2: M2 = L (G R) in bf16; S and M2 share one psz slot
            # need-ordered halves: the transpose and the (k0,i1) S MM
            # only require g_s0[:,128:256]
            nc.scalar.copy(g_s[1][:, 128:256], gps[:, 256:384])
            nc.vector.tensor_copy(g_s[0][:, 128:256], gps[:, 128:256])
            # G21 = G12^T via PE transpose (bf16 SBUF -> bf16 PSUM tile)
            nc.tensor.matmul(gt_ps[:], g_s[0][:, 128:256],
                             wl[:, 1024:1152], is_transpose=True,
                             start=True, stop=True, skip_group_check=True)
            nc.vector.tensor_copy(g_s[0][:, 0:128], gps[:, 0:128])
            s_s = bpool.tile([128, 512], BF16, tag="s_s")
            psm = psz.tile([128, 1024], F32, tag="z")
            for i in (1, 0):
                nc.tensor.matmul(psm[:, i * 512:i * 512 + 256],
                                 g_s[0][:, i * 128:(i + 1) * 128],
                                 wl[:, 0:256],
                                 start=True, stop=False)
            nc.vector.tensor_copy(g_s[1][:, 0:128], gt_ps[:])
            for i in (1, 0):
                nc.tensor.matmul(psm[:, i * 512:i * 512 + 256],
                                 g_s[1][:, i * 128:(i + 1) * 128],
                                 wl[:, 256:512],
                                 start=False, stop=True)
            # filler matmuls run while the copies drain, keeping HAM warm
            for _ in range(3):
                nc.tensor.matmul(gps[:, 0:256], wz[:, 0:128], wz[:, 0:256],
                                 start=True, stop=True, skip_group_check=True)
            nc.scalar.copy(s_s[:, 256:512], psm[:, 512:768])
            nc.vector.tensor_copy(s_s[:, 0:256], psm[:, 0:256])

            # M2 = L S: row-block m (= lhsT k-chunk for phase 3) per
            # bank; region m=0 completes first (it gates z's first MM)
            m2_s = bpool.tile([128, 512], BF16, tag="m2_s")
            for m in range(2):
                for k in (1, 0):
                    nc.tensor.matmul(
                        psm[:, m * 512:m * 512 + 256],
                        wl[:, 512 + k * 256 + m * 128:512 + k * 256 + m * 128 + 128],
                        s_s[:, k * 256:(k + 1) * 256],
                        start=(k == 1), stop=(k == 0))
            for _ in range(3):
                nc.tensor.matmul(gps[:, 256:512], wz[:, 0:128],
                                 wz[:, 0:256],
                                 start=True, stop=True, skip_group_check=True)
            # need-ordered quarter copies: the first z matmul (m=0,k=0)
            # only waits for m2_s[:,0:128]; (m=0,k=1) for [256:384]
            nc.vector.tensor_copy(m2_s[:, 0:128], psm[:, 0:128])
            nc.scalar.copy(m2_s[:, 256:384], psm[:, 512:640])
            nc.vector.tensor_copy(m2_s[:, 128:256], psm[:, 128:256])
            nc.scalar.copy(m2_s[:, 384:512], psm[:, 640:768])

            # ---- phase 3: delta^T = M2^T X^T + t2 (bf16), stream out
            # pz spans 2 PSUM banks; 2 col-blocks of 512 per drain/store
            z_s = [[bpool.tile([128, 1024], BF16, tag=f"z{m}_{q}",
                               name=f"z{m}_{q}") for q in range(3)]
                   for m in range(2)]
            z3 = [[bpool.tile([128, 512], BF16, tag=f"z3{m}_{h}",
                              name=f"z3{m}_{h}") for h in range(2)]
                  for m in range(2)]
            # spread stores over three queues so receipt latencies and
            # per-instruction costs overlap; sync is kept free late so
            # the final splits land on an idle queue
            STORE_ENG = {(0, 0): "sync", (0, 1): "scalar",
                         (1, 0): "gpsimd", (1, 1): "sync",
                         (2, 0): "scalar", (2, 1): "gpsimd"}
            for q in range(4):
                for m in range(2):
                    pz = psz.tile([128, 1024], F32, tag="z")
                    for j in range(2):
                        nc.tensor.matmul(
                            pz[:, j * 512:(j + 1) * 512],
                            m2_s[:, m * 128:(m + 1) * 128],
                            xq[q][:, j * 512:j * 512 + 512],
                            start=True, stop=False)
                        nc.tensor.matmul(
                            pz[:, j * 512:(j + 1) * 512],
                            m2_s[:, 256 + m * 128:256 + (m + 1) * 128],
                            xq[q][:, 1024 + j * 512:1024 + j * 512 + 512],
                            start=False, stop=True)
                    out_d = dt_d[q * 2 + m]
                    if q == 3:
                        # final quarter: independent half tiles so the
                        # DVE and ACT drains truly run in parallel
                        dst = None
                        nc.vector.tensor_scalar_add(z3[m][0][:],
                                                    pz[:, 0:512],
                                                    t2c[:, m:m + 1])
                        nc.scalar.activation(
                            z3[m][1][:], pz[:, 512:1024],
                            mybir.ActivationFunctionType.Identity,
                            bias=t2c[:, m:m + 1])
                    elif (q * 2 + m) % 2 == 0:
                        dst = z_s[m][q]
                        nc.vector.tensor_scalar_add(dst[:], pz[:],
                                                    t2c[:, m:m + 1])
                    else:
                        dst = z_s[m][q]
                        nc.scalar.activation(
                            dst[:], pz[:],
                            mybir.ActivationFunctionType.Identity,
                            bias=t2c[:, m:m + 1])
                    if q == 3:
                        # final stores spread over three queues
                        eng0 = nc.sync if m == 0 else nc.gpsimd
                        eng1 = nc.scalar if m == 0 else nc.sync
                        eng0.dma_start(out_d[:, 0:512], z3[m][0][:])
                        eng1.dma_start(out_d[:, 512:1024], z3[m][1][:])
                    else:
                        eng = getattr(nc, STORE_ENG[(q, m)])
                        eng.dma_start(out_d[:], dst[:])

    nc.compile()
    return nc


def _get_nc(mode=DEFAULT_MODE):
    key = ("nc", mode)
    if key not in _CACHE:
        _CACHE[key] = _build_nc(mode)
    return _CACHE[key]


def _fold_params(w_g, b_g, w_theta, b_theta, w_phi, b_phi, w_W, b_W,
                 bn_gamma, bn_beta, bn_mean, bn_var):
    f32 = np.float32
    bf = ml_dtypes.bfloat16
    s = (bn_gamma / np.sqrt(bn_var + BN_EPS)).astype(f32)
    t2 = ((b_W - bn_mean) * s + bn_beta).astype(f32)
    L = (np.asarray(w_theta, f32) @ np.asarray(w_phi, f32).T).astype(f32)
    R = (np.asarray(w_g, f32) @ (np.asarray(w_W, f32) * s[None, :]) / N).astype(f32)
    # wl[:, :512][p, k*256+j] = R[k*128+p, j]; wl[:, 512:1024] likewise
    # for L^T; wl[:, 1024:1152] = identity (PE-transpose operand)
    pack = lambda M: M.reshape(2, 128, 256).transpose(1, 0, 2).reshape(128, 512)
    wl = np.ascontiguousarray(
        np.concatenate([pack(R), pack(np.ascontiguousarray(L.T)),
                        np.eye(128, dtype=f32)], axis=1).astype(bf))
    t2c = np.ascontiguousarray(t2.reshape(2, 128).T, dtype=f32)
    return wl, t2c


def _reference_fallback(x, w_g, b_g, w_theta, b_theta, w_phi, b_phi, w_W, b_W,
                        bn_gamma, bn_beta, bn_mean, bn_var):
    b, h, w, c = x.shape
    n = h * w
    xf = x.reshape(b, n, c).astype(np.float32)
    g_x = xf @ w_g + b_g
    theta_x = xf @ w_theta + b_theta
    phi_x = xf @ w_phi + b_phi
    a = np.einsum("bnd,bne->bde", phi_x, g_x) / n
    y = theta_x @ a
    w_y = y @ w_W + b_W
    w_y = bn_gamma * (w_y - bn_mean) / np.sqrt(bn_var + BN_EPS) + bn_beta
    return (w_y.reshape(b, h, w, c) + x).astype(np.float32)


def run_sharded(x, folded, mode=DEFAULT_MODE, trace=False):
    from concourse.bass_utils import run_bass_kernel_spmd

    nc = _get_nc(mode)
    wl, t2c = folded
    bf = ml_dtypes.bfloat16
    f8 = ml_dtypes.float8_e4m3
    xf = np.asarray(x, dtype=np.float32).reshape(B, N, C)
    # x8[p, t*256+c] = x[t*128+p, c], then split into contiguous pieces
    x8 = (np.clip(xf, -240.0, 240.0).reshape(B, 32, 128, 256)
          .transpose(0, 2, 1, 3).reshape(B, 128, 8192).astype(f8))
    x8p = [np.ascontiguousarray(x8[:, :, c0 * 256:(c0 + n) * 256])
           for (c0, n, r) in XSPLIT]
    # xq[q][p, k*1024+j] = X^T[k*128+p, q*1024+j]
    xt = xf.transpose(0, 2, 1).reshape(B, 2, 128, 4, 1024).astype(bf)
    xqp = [np.ascontiguousarray(
        xt[:, :, :, q].transpose(0, 2, 1, 3).reshape(B, 128, 2048))
        for q in range(4)]
    in_maps = []
    for i in range(B):
        m = {"wl": wl, "t2c": t2c}
        for u in range(len(XSPLIT)):
            m[f"x8_{u}"] = x8p[u][i]
        for q in range(4):
            m[f"xq_{q}"] = xqp[q][i]
        in_maps.append(m)
    res = run_bass_kernel_spmd(nc, in_maps, list(range(B)), trace=trace)
    # reassemble delta^T chunks: dt_{qm} = delta^T[m*128:(m+1)*128, q*1024:...]
    z = xf.copy()
    for i in range(B):
        dT = np.empty((C, N), np.float32)
        for q in range(4):
            for m in range(2):
                dT[m * 128:(m + 1) * 128, q * 1024:(q + 1) * 1024] = \
                    np.asarray(res.results[i][f"dt_{q}{m}"], np.float32)
        z[i] += dT.T
    return np.ascontiguousarray(z.reshape(B, H, W, C)), res


def kernel(x, w_g, b_g, w_theta, b_theta, w_phi, b_phi, w_W, b_W,
           bn_gamma, bn_beta, bn_mean, bn_var):
    args = dict(w_g=np.asarray(w_g), b_g=np.asarray(b_g),
                w_theta=np.asarray(w_theta), b_theta=np.asarray(b_theta),
                w_phi=np.asarray(w_phi), b_phi=np.asarray(b_phi),
                w_W=np.asarray(w_W), b_W=np.asarray(b_W),
                bn_gamma=np.asarray(bn_gamma), bn_beta=np.asarray(bn_beta),
                bn_mean=np.asarray(bn_mean), bn_var=np.asarray(bn_var))
    x = np.asarray(x)
    # the device path folds the (zero) projection biases away; anything else
    # (never produced by setup_inputs) gets the exact host fallback
    if (np.any(args["b_g"]) or np.any(args["b_theta"]) or np.any(args["b_phi"])
            or x.shape != (B, H, W, C)):
        return _reference_fallback(x, **{k: v for k, v in args.items()})
    folded = _fold_params(**args)
    z, _ = run_sharded(x, folded)
    return z


# revision 34
# speedup vs baseline: 1.0050x; 1.0050x over previous
"""Non-local block (no softmax) on 8 Trainium2 cores, data-parallel over batch.

Math: per sample X [N=4096, C=256] (N = 64*64 spatial, C channels):
    theta = X Wt, phi = X Wp, g = X Wg          (biases are zero)
    y = (theta phi^T / N) g  ->  associativity (no softmax):
      y = X L G R,   L = Wt Wp^T,  R = Wg (Ww*s) / N,  G = X^T X
    z = y + t2 + X,  s = gamma*rsqrt(var+eps),  t2 = (b_W - mean)*s + beta
Device computes delta^T = (L G R)^T X^T + t2 in bf16; host adds X (f32).

With all 8 cores running, the kernel is HBM-wire-bound: ~5.3MB/core of
traffic paces everything.  v5 therefore packs EVERY transfer as a fully
contiguous DRAM block (one tensor per transfer, host pre/post-packs) so
HBM sees sequential streams, keeps the transfer count minimal, spreads
the load over 3 queues (sync/scalar HWDGE + gpsimd SWDGE), uses the
symmetry of G (384 instead of 512 PSUM columns/chunk + one PE
transpose), and bridges the HAM cold window with dummy matmuls.
"""

import numpy as np
import ml_dtypes

B, H, W, C = 8, 64, 64, 256
IC = C // 2
N = H * W
NCHUNK = N // 128  # 32
BN_EPS = 1e-3

_CACHE = {}
DEFAULT_MODE = "v9"
N_WARMUP = 4
# x8 pieces (chunk ranges) in consumption order, all on the sync queue:
# concurrent DMA queues interleave packets and wreck HBM streaming (2MB
# on 1 queue costs +2.4us vs +5.1us on 2 queues, measured), so a single
# queue carries everything in consumption order.
XSPLIT = [(0, 4, 0), (4, 8, 0), (12, 10, 0), (22, 10, 0)]


def _build_nc(mode: str):
    import concourse.bacc as bacc
    import concourse.mybir as mybir
    import concourse.tile as tile

    F32 = mybir.dt.float32
    BF16 = mybir.dt.bfloat16
    FP8 = mybir.dt.float8e4

    nc = bacc.Bacc("TRN2", target_bir_lowering=False, debug=False)

    # one contiguous DRAM tensor per transfer
    x8_d = [nc.dram_tensor(f"x8_{u}", [128, 256 * n], FP8,
                           kind="ExternalInput")
            for u, (c0, n, r) in enumerate(XSPLIT)]
    # xq_d[q][p, k*1024+j] = X^T[k*128+p, q*1024+j]
    xq_d = [nc.dram_tensor(f"xq_{q}", [128, 2048], BF16,
                           kind="ExternalInput") for q in range(4)]
    # wl: [0:512] R-pack, [512:1024] L^T-pack, [1024:1152] identity
    wl_d = nc.dram_tensor("wl", [128, 1152], BF16, kind="ExternalInput")
    t2c_d = nc.dram_tensor("t2c", [128, 2], F32, kind="ExternalInput")
    # output: one contiguous chunk per (q, m): delta^T[m-rows, q-cols]
    dt_d = [nc.dram_tensor(f"dt_{q}{m}", [128, 1024], BF16,
                           kind="ExternalOutput")
            for q in range(4) for m in range(2)]

    with tile.TileContext(nc) as tc:
        with (
            tc.tile_pool(name="const", bufs=1) as cpool,
            tc.tile_pool(name="big", bufs=1) as bpool,
            tc.tile_pool(name="psg", bufs=1, space="PSUM") as psg,
            tc.tile_pool(name="psz", bufs=3, space="PSUM") as psz,
        ):
            wl = cpool.tile([128, 1152], BF16, tag="wl")
            t2c = cpool.tile([128, 2], F32, tag="t2c")
            wz = cpool.tile([128, 512], BF16, tag="wz")
            # gpsimd is otherwise idle and starts earliest
            nc.gpsimd.memset(wz[:], 0.0)

            x8_t = [bpool.tile([128, 256 * n], FP8, tag=f"x8_{u}",
                               name=f"x8_{u}")
                    for u, (c0, n, r) in enumerate(XSPLIT)]
            xq = [bpool.tile([128, 2048], BF16, tag=f"xq{q}",
                             name=f"xq{q}") for q in range(4)]

            # ---- input DMAs: ALL on the sync queue, consumption order
            for u, (c0, n, r) in enumerate(XSPLIT):
                nc.sync.dma_start(x8_t[u][:], x8_d[u][:])
            nc.sync.dma_start(xq[0][:], xq_d[0][:])
            nc.sync.dma_start(wl[:], wl_d[:])
            for q in range(1, 4):
                nc.sync.dma_start(xq[q][:], xq_d[q][:])
            nc.sync.dma_start(t2c[:], t2c_d[:])

            # dummies bridge PE from preamble end to the first x8 piece
            # landing so HAM warmth builds with no idle gap
            wu = psz.tile([128, 1024], F32, tag="z")
            for _ in range(N_WARMUP):
                nc.tensor.matmul(wu[:, 0:512], wz[:, 0:128], wz[:],
                                 start=True, stop=True, skip_group_check=True)

            # ---- phase 1: G = X^T X (fp8, streams behind the x8 DMAs).
            # By symmetry only G[0:128, 0:256] (-> gps[:,0:256]) and
            # G[128:256, 128:256] (-> gps[:,256:384]) are computed; G21
            # comes from a PE transpose of G12.  The row blocks share
            # one PSUM bank: the first matmul's start=True clears
            # has_written for the whole bank, the second group opens
            # with start=False (overwrite where has_written==0).
            gps = psg.tile([128, 512], F32, tag="gp")
            gt_ps = psg.tile([128, 128], BF16, tag="gt")
            g_s = [bpool.tile([128, 256], BF16, tag=f"g_s{j}", name=f"g_s{j}")
                   for j in range(2)]
            for t in range(NCHUNK):
                u = next(i for i, (c0, n, r) in enumerate(XSPLIT)
                         if c0 <= t < c0 + n)
                xn = x8_t[u]
                o = (t - XSPLIT[u][0]) * 256
                nc.tensor.matmul(gps[:, 0:256], xn[:, o:o + 128],
                                 xn[:, o:o + 256],
                                 start=(t == 0), stop=(t == NCHUNK - 1),
                                 skip_group_check=True)
                nc.tensor.matmul(gps[:, 256:384], xn[:, o + 128:o + 256],
                                 xn[:, o + 128:o + 256],
                                 start=False, stop=(t == NCHUNK - 1),
                                 skip_group_check=True)

            # ---- phase 2: M2 = L (G R) in bf16; S and M2 share one psz slot
            # need-ordered halves: the transpose and the (k0,i1) S MM
            # only require g_s0[:,128:256]
            nc.scalar.copy(g_s[1][:, 128:256], gps[:, 256:384])
            nc.vector.tensor_copy(g_s[0][:, 128:256], gps[:, 128:256])
            # G21 = G12^T via PE transpose (bf16 SBUF -> bf16 PSUM tile)
            nc.tensor.matmul(gt_ps[:], g_s[0][:, 128:256],
                             wl[:, 1024:1152], is_transpose=True,
                             start=True, stop=True, skip_group_check=True)
            nc.vector.tensor_copy(g_s[0][:, 0:128], gps[:, 0:128])
            s_s = bpool.tile([128, 512], BF16, tag="s_s")
            psm = psz.tile([128, 1024], F32, tag="z")
            for i in (1, 0):
                nc.tensor.matmul(psm[:, i * 512:i * 512 + 256],
                                 g_s[0][:, i * 128:(i + 1) * 128],
                                 wl[:, 0:256],
                                 start=True, stop=False)
            nc.vector.tensor_copy(g_s[1][:, 0:128], gt_ps[:])
            for i in (1, 0):
                nc.tensor.matmul(psm[:, i * 512:i * 512 + 256],
                                 g_s[1][:, i * 128:(i + 1) * 128],
                                 wl[:, 256:512],
                                 start=False, stop=True)
            # filler matmuls run while the copies drain, keeping HAM warm
            for _ in range(3):
                nc.tensor.matmul(gps[:, 0:256], wz[:, 0:128], wz[:, 0:256],
                                 start=True, stop=True, skip_group_check=True)
            nc.scalar.copy(s_s[:, 256:512], psm[:, 512:768])
            nc.vector.tensor_copy(s_s[:, 0:256], psm[:, 0:256])

            # M2 = L S: row-block m (= lhsT k-chunk for phase 3) per
            # bank; region m=0 completes first (it gates z's first MM)
            m2_s = bpool.tile([128, 512], BF16, tag="m2_s")
            for m in range(2):
                for k in (1, 0):
                    nc.tensor.matmul(
                        psm[:, m * 512:m * 512 + 256],
                        wl[:, 512 + k * 256 + m * 128:512 + k * 256 + m * 128 + 128],
                        s_s[:, k * 256:(k + 1) * 256],
                        start=(k == 1), stop=(k == 0))
            for _ in range(3):
                nc.tensor.matmul(gps[:, 256:512], wz[:, 0:128],
                                 wz[:, 0:256],
                                 start=True, stop=True, skip_group_check=True)
            # need-ordered quarter copies: the first z matmul (m=0,k=0)
            # only waits for m2_s[:,0:128]; (m=0,k=1) for [256:384]
            nc.vector.tensor_copy(m2_s[:, 0:128], psm[:, 0:128])
            nc.scalar.copy(m2_s[:, 256:384], psm[:, 512:640])
            nc.vector.tensor_copy(m2_s[:, 128:256], psm[:, 128:256])
            nc.scalar.copy(m2_s[:, 384:512], psm[:, 640:768])

            # ---- phase 3: delta^T = M2^T X^T + t2 (bf16), stream out
            # pz spans 2 PSUM banks; 2 col-blocks of 512 per drain/store
            z_s = [[bpool.tile([128, 1024], BF16, tag=f"z{m}_{q}",
                               name=f"z{m}_{q}") for q in range(3)]
                   for m in range(2)]
            z3 = [[bpool.tile([128, 512], BF16, tag=f"z3{m}_{h}",
                              name=f"z3{m}_{h}") for h in range(2)]
                  for m in range(2)]
            # spread stores over three queues so receipt latencies and
            # per-instruction costs overlap; sync is kept free late so
            # the final splits land on an idle queue
            STORE_ENG = {(0, 0): "sync", (0, 1): "scalar",
                         (1, 0): "gpsimd", (1, 1): "sync",
                         (2, 0): "scalar", (2, 1): "gpsimd"}
            for q in range(4):
                for m in range(2):
                    pz = psz.tile([128, 1024], F32, tag="z")
                    for j in range(2):
                        nc.tensor.matmul(
                            pz[:, j * 512:(j + 1) * 512],
                            m2_s[:, m * 128:(m + 1) * 128],
                            xq[q][:, j * 512:j * 512 + 512],
                            start=True, stop=False)
                        nc.tensor.matmul(
                            pz[:, j * 512:(j + 1) * 512],
                            m2_s[:, 256 + m * 128:256 + (m + 1) * 128],
                            xq[q][:, 1024 + j * 512:1024 + j * 512 + 512],
                            start=False, stop=True)
                    out_d = dt_d[q * 2 + m]
                    if q == 3:
                        # final quarter: independent half tiles so the
                        # DVE and ACT drains truly run in parallel
                        dst = None
                        nc.vector.tensor_scalar_add(z3[m][0][:],
                                                    pz[:, 0:512],
                                                    t2c[:, m:m + 1])
                        nc.scalar.activation(
                            z3[m][1][:], pz[:, 512:1024],
                            mybir.ActivationFunctionType.Identity,
                            bias=t2c[:, m:m + 1])
                    elif (q * 2 + m) % 2 == 0:
                        dst = z_s[m][q]
                        nc.vector.tensor_scalar_add(dst[:], pz[:],
                                                    t2c[:, m:m + 1])
                    else:
                        dst = z_s[m][q]
                        nc.scalar.activation(
                            dst[:], pz[:],
                            mybir.ActivationFunctionType.Identity,
                            bias=t2c[:, m:m + 1])
                    if q == 3:
                        # final stores spread over three queues
                        eng0 = nc.sync if m == 0 else nc.gpsimd
                        eng1 = nc.scalar if m == 0 else nc.sync
                        eng0.dma_start(out_d[:, 0:512], z3[m][0][:])
                        eng1.dma_start(out_d[:, 512:1024], z3[m][1][:])
                    else:
                        eng = getattr(nc, STORE_ENG[(q, m)])
                        eng.dma_start(out_d[:], dst[:])

    nc.compile()
    return nc


def _get_nc(mode=DEFAULT_MODE):
    key = ("nc", mode)
    if key not in _CACHE:
        _CACHE[key] = _build_nc(mode)
    return _CACHE[key]


def _fold_params(w_g, b_g, w_theta, b_theta, w_phi, b_phi, w_W, b_W,
                 bn_gamma, bn_beta, bn_mean, bn_var):
    f32 = np.float32
    bf = ml_dtypes.bfloat16
    s = (bn_gamma / np.sqrt(bn_var + BN_EPS)).astype(f32)
    t2 = ((b_W - bn_mean) * s + bn_beta).astype(f32)
    L = (np.asarray(w_theta, f32) @ np.asarray(w_phi, f32).T).astype(f32)
    R = (np.asarray(w_g, f32) @ (np.asarray(w_W, f32) * s[None, :]) / N).astype(f32)
    # wl[:, :512][p, k*256+j] = R[k*128+p, j]; wl[:, 512:1024] likewise
    # for L^T; wl[:, 1024:1152] = identity (PE-transpose operand)
    pack = lambda M: M.reshape(2, 128, 256).transpose(1, 0, 2).reshape(128, 512)
    wl = np.ascontiguousarray(
        np.concatenate([pack(R), pack(np.ascontiguousarray(L.T)),
                        np.eye(128, dtype=f32)], axis=1).astype(bf))
    t2c = np.ascontiguousarray(t2.reshape(2, 128).T, dtype=f32)
    return wl, t2c


def _reference_fallback(x, w_g, b_g, w_theta, b_theta, w_phi, b_phi, w_W, b_W,
                        bn_gamma, bn_beta, bn_mean, bn_var):
    b, h, w, c = x.shape
    n = h * w
    xf = x.reshape(b, n, c).astype(np.float32)
    g_x = xf @ w_g + b_g
    theta_x = xf @ w_theta + b_theta
    phi_x = xf @ w_phi + b_phi
    a = np.einsum("bnd,bne->bde", phi_x, g_x) / n
    y = theta_x @ a
    w_y = y @ w_W + b_W
    w_y = bn_gamma * (w_y - bn_mean) / np.sqrt(bn_var + BN_EPS) + bn_beta
    return (w_y.reshape(b, h, w, c) + x).astype(np.float32)


def run_sharded(x, folded, mode=DEFAULT_MODE, trace=False):
    from concourse.bass_utils import run_bass_kernel_spmd

    nc = _get_nc(mode)
    wl, t2c = folded
    bf = ml_dtypes.bfloat16
    f8 = ml_dtypes.float8_e4m3
    xf = np.asarray(x, dtype=np.float32).reshape(B, N, C)
    # x8[p, t*256+c] = x[t*128+p, c], then split into contiguous pieces
    x8 = (np.clip(xf, -240.0, 240.0).reshape(B, 32, 128, 256)
          .transpose(0, 2, 1, 3).reshape(B, 128, 8192).astype(f8))
    x8p = [np.ascontiguousarray(x8[:, :, c0 * 256:(c0 + n) * 256])
           for (c0, n, r) in XSPLIT]
    # xq[q][p, k*1024+j] = X^T[k*128+p, q*1024+j]
    xt = xf.transpose(0, 2, 1).reshape(B, 2, 128, 4, 1024).astype(bf)
    xqp = [np.ascontiguousarray(
        xt[:, :, :, q].transpose(0, 2, 1, 3).reshape(B, 128, 2048))
        for q in range(4)]
    in_maps = []
    for i in range(B):
        m = {"wl": wl, "t2c": t2c}
        for u in range(len(XSPLIT)):
            m[f"x8_{u}"] = x8p[u][i]
        for q in range(4):
            m[f"xq_{q}"] = xqp[q][i]
        in_maps.append(m)
    res = run_bass_kernel_spmd(nc, in_maps, list(range(B)), trace=trace)
    # reassemble delta^T chunks: dt_{qm} = delta^T[m*128:(m+1)*128, q*1024:...]
    z = xf.copy()
    for i in range(B):
        dT = np.empty((C, N), np.float32)
        for q in range(4):
            for m in range(2):
                dT[m * 128:(m + 1) * 128, q * 1024:(q + 1) * 1024] = \
                    np.asarray(res.results[i][f"dt_{q}{m}"], np.float32)
        z[i] += dT.T
    return np.ascontiguousarray(z.reshape(B, H, W, C)), res


def kernel(x, w_g, b_g, w_theta, b_theta, w_phi, b_phi, w_W, b_W,
           bn_gamma, bn_beta, bn_mean, bn_var):
    args = dict(w_g=np.asarray(w_g), b_g=np.asarray(b_g),
                w_theta=np.asarray(w_theta), b_theta=np.asarray(b_theta),
                w_phi=np.asarray(w_phi), b_phi=np.asarray(b_phi),
                w_W=np.asarray(w_W), b_W=np.asarray(b_W),
                bn_gamma=np.asarray(bn_gamma), bn_beta=np.asarray(bn_beta),
                bn_mean=np.asarray(bn_mean), bn_var=np.asarray(bn_var))
    x = np.asarray(x)
    # the device path folds the (zero) projection biases away; anything else
    # (never produced by setup_inputs) gets the exact host fallback
    if (np.any(args["b_g"]) or np.any(args["b_theta"]) or np.any(args["b_phi"])
            or x.shape != (B, H, W, C)):
        return _reference_fallback(x, **{k: v for k, v in args.items()})
    folded = _fold_params(**args)
    z, _ = run_sharded(x, folded)
    return z


# revision 37
# speedup vs baseline: 1.0375x; 1.0324x over previous
"""Non-local block (no softmax) on 8 Trainium2 cores, data-parallel over batch.

Math: per sample X [N=4096, C=256] (N = 64*64 spatial, C channels):
    theta = X Wt, phi = X Wp, g = X Wg          (biases are zero)
    y = (theta phi^T / N) g  ->  associativity (no softmax):
      y = X L G R,   L = Wt Wp^T,  R = Wg (Ww*s) / N,  G = X^T X
    z = y + t2 + X,  s = gamma*rsqrt(var+eps),  t2 = (b_W - mean)*s + beta
Device computes delta^T = (L G R)^T X^T + t2 in bf16; host adds X (f32).

With all 8 cores running, the kernel is HBM-wire-bound: ~5.3MB/core of
traffic paces everything.  v5 therefore packs EVERY transfer as a fully
contiguous DRAM block (one tensor per transfer, host pre/post-packs) so
HBM sees sequential streams, keeps the transfer count minimal, spreads
the load over 3 queues (sync/scalar HWDGE + gpsimd SWDGE), uses the
symmetry of G (384 instead of 512 PSUM columns/chunk + one PE
transpose), and bridges the HAM cold window with dummy matmuls.
"""

import numpy as np
import ml_dtypes

B, H, W, C = 8, 64, 64, 256
IC = C // 2
N = H * W
NCHUNK = N // 128  # 32
BN_EPS = 1e-3

_CACHE = {}
DEFAULT_MODE = "v9"
N_WARMUP = 5
# x8 pieces (chunk ranges) in consumption order, all on the sync queue:
# concurrent DMA queues interleave packets and wreck HBM streaming (2MB
# on 1 queue costs +2.4us vs +5.1us on 2 queues, measured), so a single
# queue carries everything in consumption order.
XSPLIT = [(0, 4, 0), (4, 8, 0), (12, 10, 0), (22, 10, 0)]


def _build_nc(mode: str):
    import concourse.bacc as bacc
    import concourse.mybir as mybir
    import concourse.tile as tile

    F32 = mybir.dt.float32
    BF16 = mybir.dt.bfloat16
    FP8 = mybir.dt.float8e4

    nc = bacc.Bacc("TRN2", target_bir_lowering=False, debug=False)

    # one contiguous DRAM tensor per transfer
    x8_d = [nc.dram_tensor(f"x8_{u}", [128, 256 * n], FP8,
                           kind="ExternalInput")
            for u, (c0, n, r) in enumerate(XSPLIT)]
    # xq_d[q][p, k*1024+j] = X^T[k*128+p, q*1024+j]
    xq_d = [nc.dram_tensor(f"xq_{q}", [128, 2048], BF16,
                           kind="ExternalInput") for q in range(4)]
    # wl: [0:512] R-pack, [512:1024] L^T-pack, [1024:1152] identity
    wl_d = nc.dram_tensor("wl", [128, 1152], BF16, kind="ExternalInput")
    t2c_d = nc.dram_tensor("t2c", [128, 2], F32, kind="ExternalInput")
    # output: one contiguous chunk per (q, m): delta^T[m-rows, q-cols]
    dt_d = [nc.dram_tensor(f"dt_{q}{m}", [128, 1024], BF16,
                           kind="ExternalOutput")
            for q in range(4) for m in range(2)]

    with tile.TileContext(nc) as tc:
        with (
            tc.tile_pool(name="const", bufs=1) as cpool,
            tc.tile_pool(name="big", bufs=1) as bpool,
            tc.tile_pool(name="psg", bufs=1, space="PSUM") as psg,
            tc.tile_pool(name="psz", bufs=3, space="PSUM") as psz,
        ):
            wl = cpool.tile([128, 1152], BF16, tag="wl")
            t2c = cpool.tile([128, 2], F32, tag="t2c")
            wz = cpool.tile([128, 512], BF16, tag="wz")
            # gpsimd is otherwise idle and starts earliest
            nc.gpsimd.memset(wz[:], 0.0)

            x8_t = [bpool.tile([128, 256 * n], FP8, tag=f"x8_{u}",
                               name=f"x8_{u}")
                    for u, (c0, n, r) in enumerate(XSPLIT)]
            xq = [bpool.tile([128, 2048], BF16, tag=f"xq{q}",
                             name=f"xq{q}") for q in range(4)]

            # ---- input DMAs: ALL on the sync queue, consumption order
            for u, (c0, n, r) in enumerate(XSPLIT):
                nc.sync.dma_start(x8_t[u][:], x8_d[u][:])
            nc.sync.dma_start(xq[0][:], xq_d[0][:])
            nc.sync.dma_start(wl[:], wl_d[:])
            for q in range(1, 4):
                nc.sync.dma_start(xq[q][:], xq_d[q][:])
            nc.sync.dma_start(t2c[:], t2c_d[:])

            # dummies bridge PE from preamble end to the first x8 piece
            # landing so HAM warmth builds with no idle gap
            wu = psz.tile([128, 1024], F32, tag="z")
            for _ in range(N_WARMUP):
                nc.tensor.matmul(wu[:, 0:512], wz[:, 0:128], wz[:],
                                 start=True, stop=True, skip_group_check=True)

            # ---- phase 1: G = X^T X (fp8, streams behind the x8 DMAs).
            # By symmetry only G[0:128, 0:256] (-> gps[:,0:256]) and
            # G[128:256, 128:256] (-> gps[:,256:384]) are computed; G21
            # comes from a PE transpose of G12.  The row blocks share
            # one PSUM bank: the first matmul's start=True clears
            # has_written for the whole bank, the second group opens
            # with start=False (overwrite where has_written==0).
            gps = psg.tile([128, 512], F32, tag="gp")
            gt_ps = psg.tile([128, 128], BF16, tag="gt")
            g_s = [bpool.tile([128, 256], BF16, tag=f"g_s{j}", name=f"g_s{j}")
                   for j in range(2)]
            for t in range(NCHUNK):
                u = next(i for i, (c0, n, r) in enumerate(XSPLIT)
                         if c0 <= t < c0 + n)
                xn = x8_t[u]
                o = (t - XSPLIT[u][0]) * 256
                nc.tensor.matmul(gps[:, 0:256], xn[:, o:o + 128],
                                 xn[:, o:o + 256],
                                 start=(t == 0), stop=(t == NCHUNK - 1),
                                 skip_group_check=True)
                nc.tensor.matmul(gps[:, 256:384], xn[:, o + 128:o + 256],
                                 xn[:, o + 128:o + 256],
                                 start=False, stop=(t == NCHUNK - 1),
                                 skip_group_check=True)

            # ---- phase 2: M2 = L (G R) in bf16; S and M2 share one psz slot
            # need-ordered halves: the transpose and the (k0,i1) S MM
            # only require g_s0[:,128:256]
            nc.scalar.copy(g_s[1][:, 128:256], gps[:, 256:384])
            nc.vector.tensor_copy(g_s[0][:, 128:256], gps[:, 128:256])
            # G21 = G12^T via PE transpose (bf16 SBUF -> bf16 PSUM tile)
            nc.tensor.matmul(gt_ps[:], g_s[0][:, 128:256],
                             wl[:, 1024:1152], is_transpose=True,
                             start=True, stop=True, skip_group_check=True)
            nc.vector.tensor_copy(g_s[0][:, 0:128], gps[:, 0:128])
            s_s = bpool.tile([128, 512], BF16, tag="s_s")
            psm = psz.tile([128, 1024], F32, tag="z")
            for i in (1, 0):
                nc.tensor.matmul(psm[:, i * 512:i * 512 + 256],
                                 g_s[0][:, i * 128:(i + 1) * 128],
                                 wl[:, 0:256],
                                 start=True, stop=False)
            nc.vector.tensor_copy(g_s[1][:, 0:128], gt_ps[:])
            for i in (1, 0):
                nc.tensor.matmul(psm[:, i * 512:i * 512 + 256],
                                 g_s[1][:, i * 128:(i + 1) * 128],
                                 wl[:, 256:512],
                                 start=False, stop=True)
            # filler matmuls run while the copies drain, keeping HAM warm
            for _ in range(3):
                nc.tensor.matmul(gps[:, 0:256], wz[:, 0:128], wz[:, 0:256],
                                 start=True, stop=True, skip_group_check=True)
            nc.scalar.copy(s_s[:, 256:512], psm[:, 512:768])
            nc.vector.tensor_copy(s_s[:, 0:256], psm[:, 0:256])

            # M2 = L S: row-block m (= lhsT k-chunk for phase 3) per
            # bank; region m=0 completes first (it gates z's first MM)
            m2_s = bpool.tile([128, 512], BF16, tag="m2_s")
            for m in range(2):
                for k in (1, 0):
                    nc.tensor.matmul(
                        psm[:, m * 512:m * 512 + 256],
                        wl[:, 512 + k * 256 + m * 128:512 + k * 256 + m * 128 + 128],
                        s_s[:, k * 256:(k + 1) * 256],
                        start=(k == 1), stop=(k == 0))
            for _ in range(3):
                nc.tensor.matmul(gps[:, 256:512], wz[:, 0:128],
                                 wz[:, 0:256],
                                 start=True, stop=True, skip_group_check=True)
            # need-ordered quarter copies: the first z matmul (m=0,k=0)
            # only waits for m2_s[:,0:128]; (m=0,k=1) for [256:384]
            nc.vector.tensor_copy(m2_s[:, 0:128], psm[:, 0:128])
            nc.scalar.copy(m2_s[:, 256:384], psm[:, 512:640])
            nc.vector.tensor_copy(m2_s[:, 128:256], psm[:, 128:256])
            nc.scalar.copy(m2_s[:, 384:512], psm[:, 640:768])

            # ---- phase 3: delta^T = M2^T X^T + t2 (bf16), stream out
            # pz spans 2 PSUM banks; 2 col-blocks of 512 per drain/store
            z_s = [[bpool.tile([128, 1024], BF16, tag=f"z{m}_{q}",
                               name=f"z{m}_{q}") for q in range(3)]
                   for m in range(2)]
            z3 = [[bpool.tile([128, 512], BF16, tag=f"z3{m}_{h}",
                              name=f"z3{m}_{h}") for h in range(2)]
                  for m in range(2)]
            # spread stores over three queues so receipt latencies and
            # per-instruction costs overlap; sync is kept free late so
            # the final splits land on an idle queue
            STORE_ENG = {(0, 0): "sync", (0, 1): "scalar",
                         (1, 0): "gpsimd", (1, 1): "sync",
                         (2, 0): "scalar", (2, 1): "gpsimd"}
            for q in range(4):
                for m in range(2):
                    pz = psz.tile([128, 1024], F32, tag="z")
                    for j in range(2):
                        nc.tensor.matmul(
                            pz[:, j * 512:(j + 1) * 512],
                            m2_s[:, m * 128:(m + 1) * 128],
                            xq[q][:, j * 512:j * 512 + 512],
                            start=True, stop=False)
                        nc.tensor.matmul(
                            pz[:, j * 512:(j + 1) * 512],
                            m2_s[:, 256 + m * 128:256 + (m + 1) * 128],
                            xq[q][:, 1024 + j * 512:1024 + j * 512 + 512],
                            start=False, stop=True)
                    out_d = dt_d[q * 2 + m]
                    if q == 3:
                        # final quarter: independent half tiles so the
                        # DVE and ACT drains truly run in parallel
                        dst = None
                        nc.vector.tensor_scalar_add(z3[m][0][:],
                                                    pz[:, 0:512],
                                                    t2c[:, m:m + 1])
                        nc.scalar.activation(
                            z3[m][1][:], pz[:, 512:1024],
                            mybir.ActivationFunctionType.Identity,
                            bias=t2c[:, m:m + 1])
                    elif (q * 2 + m) % 2 == 0:
                        dst = z_s[m][q]
                        nc.vector.tensor_scalar_add(dst[:], pz[:],
                                                    t2c[:, m:m + 1])
                    else:
                        dst = z_s[m][q]
                        nc.scalar.activation(
                            dst[:], pz[:],
                            mybir.ActivationFunctionType.Identity,
                            bias=t2c[:, m:m + 1])
                    if q == 3:
                        # final stores spread over three queues
                        eng0 = nc.sync if m == 0 else nc.gpsimd
                        eng1 = nc.scalar if m == 0 else nc.sync
                        eng0.dma_start(out_d[:, 0:512], z3[m][0][:])
                        eng1.dma_start(out_d[:, 512:1024], z3[m][1][:])
                    else:
                        eng = getattr(nc, STORE_ENG[(q, m)])
                        eng.dma_start(out_d[:], dst[:])

    nc.compile()
    return nc


def _get_nc(mode=DEFAULT_MODE):
    key = ("nc", mode)
    if key not in _CACHE:
        _CACHE[key] = _build_nc(mode)
    return _CACHE[key]


def _fold_params(w_g, b_g, w_theta, b_theta, w_phi, b_phi, w_W, b_W,
                 bn_gamma, bn_beta, bn_mean, bn_var):
    f32 = np.float32
    bf = ml_dtypes.bfloat16
    s = (bn_gamma / np.sqrt(bn_var + BN_EPS)).astype(f32)
    t2 = ((b_W - bn_mean) * s + bn_beta).astype(f32)
    L = (np.asarray(w_theta, f32) @ np.asarray(w_phi, f32).T).astype(f32)
    R = (np.asarray(w_g, f32) @ (np.asarray(w_W, f32) * s[None, :]) / N).astype(f32)
    # wl[:, :512][p, k*256+j] = R[k*128+p, j]; wl[:, 512:1024] likewise
    # for L^T; wl[:, 1024:1152] = identity (PE-transpose operand)
    pack = lambda M: M.reshape(2, 128, 256).transpose(1, 0, 2).reshape(128, 512)
    wl = np.ascontiguousarray(
        np.concatenate([pack(R), pack(np.ascontiguousarray(L.T)),
                        np.eye(128, dtype=f32)], axis=1).astype(bf))
    t2c = np.ascontiguousarray(t2.reshape(2, 128).T, dtype=f32)
    return wl, t2c


def _reference_fallback(x, w_g, b_g, w_theta, b_theta, w_phi, b_phi, w_W, b_W,
                        bn_gamma, bn_beta, bn_mean, bn_var):
    b, h, w, c = x.shape
    n = h * w
    xf = x.reshape(b, n, c).astype(np.float32)
    g_x = xf @ w_g + b_g
    theta_x = xf @ w_theta + b_theta
    phi_x = xf @ w_phi + b_phi
    a = np.einsum("bnd,bne->bde", phi_x, g_x) / n
    y = theta_x @ a
    w_y = y @ w_W + b_W
    w_y = bn_gamma * (w_y - bn_mean) / np.sqrt(bn_var + BN_EPS) + bn_beta
    return (w_y.reshape(b, h, w, c) + x).astype(np.float32)


def run_sharded(x, folded, mode=DEFAULT_MODE, trace=False):
    from concourse.bass_utils import run_bass_kernel_spmd

    nc = _get_nc(mode)
    wl, t2c = folded
    bf = ml_dtypes.bfloat16
    f8 = ml_dtypes.float8_e4m3
    xf = np.asarray(x, dtype=np.float32).reshape(B, N, C)
    # x8[p, t*256+c] = x[t*128+p, c], then split into contiguous pieces
    x8 = (np.clip(xf, -240.0, 240.0).reshape(B, 32, 128, 256)
          .transpose(0, 2, 1, 3).reshape(B, 128, 8192).astype(f8))
    x8p = [np.ascontiguousarray(x8[:, :, c0 * 256:(c0 + n) * 256])
           for (c0, n, r) in XSPLIT]
    # xq[q][p, k*1024+j] = X^T[k*128+p, q*1024+j]
    xt = xf.transpose(0, 2, 1).reshape(B, 2, 128, 4, 1024).astype(bf)
    xqp = [np.ascontiguousarray(
        xt[:, :, :, q].transpose(0, 2, 1, 3).reshape(B, 128, 2048))
        for q in range(4)]
    in_maps = []
    for i in range(B):
        m = {"wl": wl, "t2c": t2c}
        for u in range(len(XSPLIT)):
            m[f"x8_{u}"] = x8p[u][i]
        for q in range(4):
            m[f"xq_{q}"] = xqp[q][i]
        in_maps.append(m)
    res = run_bass_kernel_spmd(nc, in_maps, list(range(B)), trace=trace)
    # reassemble delta^T chunks: dt_{qm} = delta^T[m*128:(m+1)*128, q*1024:...]
    z = xf.copy()
    for i in range(B):
        dT = np.empty((C, N), np.float32)
        for q in range(4):
            for m in range(2):
                dT[m * 128:(m + 1) * 128, q * 1024:(q + 1) * 1024] = \
                    np.asarray(res.results[i][f"dt_{q}{m}"], np.float32)
        z[i] += dT.T
    return np.ascontiguousarray(z.reshape(B, H, W, C)), res


def kernel(x, w_g, b_g, w_theta, b_theta, w_phi, b_phi, w_W, b_W,
           bn_gamma, bn_beta, bn_mean, bn_var):
    args = dict(w_g=np.asarray(w_g), b_g=np.asarray(b_g),
                w_theta=np.asarray(w_theta), b_theta=np.asarray(b_theta),
                w_phi=np.asarray(w_phi), b_phi=np.asarray(b_phi),
                w_W=np.asarray(w_W), b_W=np.asarray(b_W),
                bn_gamma=np.asarray(bn_gamma), bn_beta=np.asarray(bn_beta),
                bn_mean=np.asarray(bn_mean), bn_var=np.asarray(bn_var))
    x = np.asarray(x)
    # the device path folds the (zero) projection biases away; anything else
    # (never produced by setup_inputs) gets the exact host fallback
    if (np.any(args["b_g"]) or np.any(args["b_theta"]) or np.any(args["b_phi"])
            or x.shape != (B, H, W, C)):
        return _reference_fallback(x, **{k: v for k, v in args.items()})
    folded = _fold_params(**args)
    z, _ = run_sharded(x, folded)
    return z
